# revision 34
# baseline (speedup 1.0000x reference)
"""Trainium2 Bass kernel for a pre-norm MQA decoder layer (dense_transformer).

Model (per batch element b, seq s=2048, d=4096, 32 heads x dk=128, d_ff=16384):
  xn = rmsnorm(x)*scale_attn; q,k,v = proj(xn) (MQA: single k/v head)
  attn = softmax(q k^T / sqrt(dk)) v;  x1 = x + attn @ Wo + bo
  xn2 = rmsnorm(x1)*scale_ffn;  out = x1 + gelu(xn2 @ W1 + b1) @ W2 + b2

Sharding: pure data parallel over 8 cores. Each core owns 512 query tokens
(batch be=c//4, rows (c%4)*512..+512) and redundantly computes the full
2048-token K/V for its batch element (cheap for MQA: dk=128). No collectives.
Per-core kv input is rotated so the core's own 512 tokens are always slab 0.

All matmul operands are bf16 (weights pre-cast on host); PSUM accumulation is
f32 and the residual accumulator x1T stays f32. rmsnorm is applied via raw-x
matmuls with the per-token 1/rms folded downstream:
  - k/v/q projections consume raw x^T (bf16, produced by XBAR DMA-transpose)
  - per-kv-token inv rides the Exp activation's per-partition scale AP
  - per-own-token inv is one DVE mult on q (broadcast tile from Pool engine)
  - v is scaled in token-major where inv is per-partition (fused with +bv)
  - bq is folded into the exp bias via kbias = (k^T bq) * inv * kscale
Softmax sums run off the PE: DVE-accumulated exp chunks + Pool-engine
partition_all_reduce + reciprocal_approx_fast. Wo is interleaved with
attention head-groups (8 heads -> partial Wo) to keep the PE busy while the
Act engine streams exps.
"""

import sys

if "/opt/trn_rl_repo" not in sys.path:
    sys.path.insert(0, "/opt/trn_rl_repo")

import numpy as np

P = 128
T = 512            # tokens per core
D = 4096
DC = D // P        # 32 feature chunks
DK = 128
NH = 32
S = 2048           # kv length
SC = S // P        # 16 kv chunks / token slabs
DFF = 16384
FBLK = 1024        # FFN f-block width
NFB = DFF // FBLK  # 16 f-blocks
NCORES = 8
EPS = 1e-10
KSCALE = 1.0 / float(np.sqrt(128.0))

_CACHE = {}
LAST_RESULTS = None  # test.py reads exec_time_ns from here


def _build_program():
    import concourse.tile as tile
    from concourse import bacc, bass_isa, mybir
    from concourse.masks import make_identity

    f32 = mybir.dt.float32
    bf16 = mybir.dt.bfloat16
    AF = mybir.ActivationFunctionType
    ALU = mybir.AluOpType
    RED = bass_isa.ReduceOp

    nc = bacc.Bacc("TRN2", target_bir_lowering=False, num_devices=NCORES)

    # wk/wv and the per-chunk bias views are pre-arranged on the host into
    # partition-major layouts so their DMAs are contiguous per partition
    # (the naive rearranged reads generate thousands of tiny descriptors)
    x_kv = nc.dram_tensor("x_kv", [S, D], bf16, kind="ExternalInput")
    wq = nc.dram_tensor("wq", [D, D], bf16, kind="ExternalInput")
    wk = nc.dram_tensor("wk", [P, DC * DK], bf16, kind="ExternalInput")
    wv = nc.dram_tensor("wv", [P, DC * DK], bf16, kind="ExternalInput")
    wo = nc.dram_tensor("wo", [D, D], bf16, kind="ExternalInput")
    w1 = nc.dram_tensor("w1", [D, DFF], bf16, kind="ExternalInput")
    w2 = nc.dram_tensor("w2", [DFF, D], bf16, kind="ExternalInput")
    bq = nc.dram_tensor("bq", [P, NH], f32, kind="ExternalInput")
    bk = nc.dram_tensor("bk", [DK], f32, kind="ExternalInput")
    bv = nc.dram_tensor("bv", [DK], f32, kind="ExternalInput")
    bo = nc.dram_tensor("bo", [P, DC], f32, kind="ExternalInput")
    b1 = nc.dram_tensor("b1", [P, DFF // P], f32, kind="ExternalInput")
    b2 = nc.dram_tensor("b2", [P, DC], f32, kind="ExternalInput")
    out = nc.dram_tensor("out", [T, D], f32, kind="ExternalOutput")

    lowp = nc.allow_low_precision(
        reason="bf16 matmul inputs / bf16 softmax accumulation are intended")
    with lowp, tile.TileContext(nc) as tc:
        consts = tc.alloc_tile_pool(name="consts", bufs=1)
        ident = consts.tile([P, P], f32)
        make_identity(nc, ident)
        ones_bf = consts.tile([P, 1], bf16)
        nc.vector.memset(ones_bf, 1.0)
        eps_sb = consts.tile([P, 1], f32)
        nc.vector.memset(eps_sb, EPS)
        bo_sb = consts.tile([P, DC], f32)
        nc.sync.dma_start(bo_sb, bo[:, :])
        b2_sb = consts.tile([P, DC], f32)
        nc.sync.dma_start(b2_sb, b2[:, :])
        b1_sb = consts.tile([P, DFF // P], f32)
        nc.sync.dma_start(b1_sb, b1[:, :])
        bk_sb = consts.tile([P, 1], f32)
        nc.sync.dma_start(bk_sb, bk[:][:, None])
        bv_row = consts.tile([1, DK], f32)
        nc.sync.dma_start(bv_row, bv[:][None, :])
        bv_bc = consts.tile([P, DK], f32)
        nc.gpsimd.partition_broadcast(bv_bc, bv_row)
        # bq as [dk, head] bf16 for the kbias fold
        bq_sb = consts.tile([P, NH], f32)
        nc.sync.dma_start(bq_sb, bq[:, :])
        bq_bf = consts.tile([P, NH], bf16)
        nc.vector.tensor_copy(bq_bf, bq_sb)
        # full K/V projection weights stay resident (1MB each in bf16)
        wk_sb = consts.tile([P, DC, DK], bf16)
        nc.sync.dma_start(wk_sb, wk[:, :].rearrange("p (c k) -> p c k", c=DC))
        wv_sb = consts.tile([P, DC, DK], bf16)
        nc.sync.dma_start(wv_sb, wv[:, :].rearrange("p (c k) -> p c k", c=DC))

        inv_kv = consts.tile([P, SC], f32)    # per-slab 1/rms (token-major)
        invk_kv = consts.tile([P, SC], f32)   # inv * 1/sqrt(dk)
        kbias = consts.tile([P, SC, NH], f32)  # (k^T bq) * invk, per kv chunk
        invbc = consts.tile([P, T], f32)      # own-token inv broadcast
        nb2 = consts.tile([P, T], f32)        # ffn-norm inv broadcast

        # persistent activation buffers (allocation order == release stack)
        p_x1 = tc.alloc_tile_pool(name="p_x1", bufs=1)
        x1T = p_x1.tile([P, DC, T], f32)      # residual accumulator
        p_xtb = tc.alloc_tile_pool(name="p_xtb", bufs=1)
        xTb = p_xtb.tile([P, DC, T], bf16)    # own raw x^T (matmul + residual)
        kv_out = tc.alloc_tile_pool(name="kv_out", bufs=1)
        kT = kv_out.tile([P, S], bf16)        # raw k^T (+bk): dk on partitions
        vtok = kv_out.tile([P, SC, DK], bf16)  # v token-major, scaled + bv
        p_head = tc.alloc_tile_pool(name="p_head", bufs=32)

        # ---- Phase A: per 512-token kv group: stats, DMA-transpose, K/V.
        # Group 0 == own tokens -> fills xTb. Emitted order: g0 stats +
        # transposes, Q-proj, g0 K/V, g1..g3 so the PE starts Q matmuls as
        # soon as the first xTb chunks and wq tiles land.
        def kv_stats(g, stat_p, nstat_p):
            for sub in range(4):
                slab = g * 4 + sub
                xs = stat_p.tile([P, D], bf16, tag="xs")
                nc.sync.dma_start(xs, x_kv[slab * P:(slab + 1) * P, :])
                NSUB = D // nc.vector.BN_STATS_FMAX
                stats = nstat_p.tile([P, NSUB, nc.vector.BN_STATS_DIM], f32,
                                     tag="st")
                xsg = xs.rearrange("p (n f) -> p n f", n=NSUB)
                for i in range(NSUB):
                    nc.vector.bn_stats(out=stats[:, i, :], in_=xsg[:, i, :])
                mv = nstat_p.tile([P, nc.vector.BN_AGGR_DIM], f32, tag="mv")
                nc.vector.bn_aggr(out=mv, in_=stats)
                msq = nstat_p.tile([P, 1], f32, tag="msq")
                nc.vector.tensor_mul(msq, mv[:, 0:1], mv[:, 0:1])
                nc.vector.tensor_add(msq, msq, mv[:, 1:2])
                rms = nstat_p.tile([P, 1], f32, tag="rms")
                nc.scalar.activation(rms, msq, AF.Sqrt, bias=eps_sb[:, 0:1])
                nc.vector.reciprocal(inv_kv[:, slab:slab + 1], rms)

        def kv_transposes(g, xg_pool):
            # all on the Act queue: concurrent XBAR transposes from two HWDGE
            # queues corrupt data (the tile-transpose unit is shared state),
            # and Act is otherwise idle here while SP streams weights
            chunks = []
            for c in range(DC):
                if g == 0:
                    ch = xTb[:, c, :]
                else:
                    ch = xg_pool.tile([P, T], bf16, tag="xg")
                nc.scalar.dma_start_transpose(
                    ch, x_kv[g * T:(g + 1) * T, c * P:(c + 1) * P])
                chunks.append(ch)
            return chunks

        def kv_proj(g, chunks, ps_kv, ps_tr, vt_p):
            kps = ps_kv.tile([P, T], f32, tag="kps")
            vps = ps_kv.tile([P, T], f32, tag="vps")
            for c in range(DC):
                nc.tensor.matmul(kps, wk_sb[:, c, :], chunks[c],
                                 start=(c == 0), stop=(c == DC - 1))
                nc.tensor.matmul(vps, wv_sb[:, c, :], chunks[c],
                                 start=(c == 0), stop=(c == DC - 1))
            # k stays raw (inv rides the exp scale); +bk here is exact
            nc.scalar.activation(kT[:, g * T:(g + 1) * T], kps, AF.Identity,
                                 bias=bk_sb[:, 0:1])
            vt = vt_p.tile([P, T], f32, tag="vt")
            nc.scalar.activation(vt, vps, AF.Identity)
            for q4 in range(4):
                slab = g * 4 + q4
                pt = ps_tr.tile([P, P], f32, tag="vtr")
                nc.tensor.transpose(pt, vt[:, q4 * P:(q4 + 1) * P], ident)
                # v[tok, dk] = (Wv^T x)[tok, dk] * inv[tok] + bv  (exact)
                nc.vector.scalar_tensor_tensor(
                    vtok[:, slab, :], pt, inv_kv[:, slab:slab + 1], bv_bc,
                    op0=ALU.mult, op1=ALU.add)

        with (
            tc.tile_pool(name="stat", bufs=1) as stat_p,
            tc.tile_pool(name="nstat", bufs=6) as nstat_p,
            tc.tile_pool(name="kvgrp", bufs=8) as kvgrp_p,
            tc.tile_pool(name="vt", bufs=2) as vt_p,
            tc.tile_pool(name="ps_kv", bufs=1, space="PSUM") as ps_kv,
            tc.tile_pool(name="ps_tr", bufs=2, space="PSUM") as ps_tr,
            tc.tile_pool(name="wq_s", bufs=5) as wq_p,
            tc.tile_pool(name="ps_q", bufs=4, space="PSUM") as ps_q,
            tc.tile_pool(name="smalls", bufs=1) as small_p,
        ):
            g0_chunks = kv_transposes(0, kvgrp_p)
            kv_stats(0, stat_p, nstat_p)

            # ---- Phase B: Q projection (head h == feature chunk h, dk=128)
            # g1..g3 kv groups are woven between mg blocks so their
            # transposes/stats stream while the PE grinds Q matmuls.
            kv_sched = {1: 0, 3: 1, 5: 2, 7: 3}
            q_tiles = []
            for mg in range(8):
                if mg in kv_sched:
                    g = kv_sched[mg]
                    if g == 0:
                        chunks = g0_chunks
                    else:
                        chunks = kv_transposes(g, kvgrp_p)
                        kv_stats(g, stat_p, nstat_p)
                    kv_proj(g, chunks, ps_kv, ps_tr, vt_p)
                pss = [ps_q.tile([P, T], f32, tag="q", name=f"psq{mg}_{j}")
                       for j in range(4)]
                for kc4 in range(8):
                    wb = wq_p.tile([P, 4, 512], bf16, tag="wq")
                    nc.sync.dma_start(
                        wb, wq[kc4 * 512:(kc4 + 1) * 512,
                               mg * 512:(mg + 1) * 512]
                        .rearrange("(k p) n -> p k n", p=P))
                    for k in range(4):
                        kc = kc4 * 4 + k
                        for j in range(4):
                            nc.tensor.matmul(pss[j],
                                             wb[:, k, j * P:(j + 1) * P],
                                             xTb[:, kc, :],
                                             start=(kc == 0),
                                             stop=(kc == DC - 1))
                if mg == 0:
                    # own-token inv as a [128, 512] broadcast tile (the PE
                    # transposes sit behind mg0's matmuls so they don't gate
                    # the Q stream on the g0 stats chain)
                    invT = small_p.tile([1, T], f32, tag="invT")
                    for sub in range(4):
                        inv1_ps = ps_tr.tile([1, P], f32, tag="vtr",
                                             name=f"invtr{sub}")
                        nc.tensor.transpose(inv1_ps, inv_kv[:, sub:sub + 1],
                                            ident)
                        nc.vector.tensor_copy(
                            invT[:, sub * P:(sub + 1) * P], inv1_ps)
                    nc.gpsimd.partition_broadcast(invbc, invT)
                for j in range(4):
                    m = mg * 4 + j
                    qt = p_head.tile([P, T], bf16, tag="head", name=f"q{m}")
                    # q = (Wq_s^T x) * inv[tok]; bq folded into exp bias
                    nc.vector.tensor_tensor(qt, pss[j], invbc, ALU.mult)
                    q_tiles.append(qt)

            nc.vector.tensor_scalar_mul(invk_kv, inv_kv, KSCALE)
            # kbias[kv, sc, h] = (k^T bq)[kv] * invk[kv]  (zero when bq == 0)
            kb_ps = ps_q.tile([P, SC, NH], f32, tag="q")
            for sc in range(SC):
                nc.tensor.matmul(kb_ps[:, sc, :], kT[:, sc * P:(sc + 1) * P],
                                 bq_bf, start=True, stop=True)
                nc.vector.tensor_scalar_mul(kbias[:, sc, :], kb_ps[:, sc, :],
                                            invk_kv[:, sc:sc + 1])

        # ---- Phase C+D: attention with the previous head-group's Wo matmuls
        # interleaved INSIDE each head's score/AV stream: the 2 extra Wo MMs
        # per sc absorb the Act exp latency so AV(sc-1) never stalls, and the
        # PE stays saturated. attn_h overwrites q_tiles[h] in place.
        def wo_block(pvg, jg, ps_wo, wo_p):
            """Emit DMA + psum tiles for Wo block (head-group pvg, jg); the
            32 MMs are emitted later, two per sc iteration."""
            wbs = []
            for kq in range(2):
                wb = wo_p.tile([P, 4, 512], bf16, tag="wob")
                r0 = (pvg * 8 + kq * 4) * P
                nc.sync.dma_start(
                    wb, wo[r0:r0 + 512, jg * 512:(jg + 1) * 512]
                    .rearrange("(k p) n -> p k n", p=P))
                wbs.append(wb)
            pss = [ps_wo.tile([P, T], f32, tag="wo",
                              name=f"pswo{pvg}_{jg}_{j}") for j in range(4)]
            return wbs, pss

        def wo_mm(pvg, jg, wbs, pss, i, q_tiles):
            """i-th of the 32 MMs of a Wo block (kc8-major, j-minor)."""
            kc8, j = i // 4, i % 4
            nc.tensor.matmul(pss[j], wbs[kc8 // 4][:, kc8 % 4,
                                                   j * P:(j + 1) * P],
                             q_tiles[pvg * 8 + kc8],
                             start=(kc8 == 0), stop=(kc8 == 7))

        def wo_accum(pvg, jg, pss):
            for j in range(4):
                c = jg * 4 + j
                if pvg == 0:
                    nc.vector.tensor_tensor(x1T[:, c, :], pss[j],
                                            xTb[:, c, :], ALU.add)
                elif pvg < 3:
                    nc.vector.tensor_tensor(x1T[:, c, :], pss[j],
                                            x1T[:, c, :], ALU.add)
                else:
                    nc.vector.scalar_tensor_tensor(
                        x1T[:, c, :], pss[j], bo_sb[:, c:c + 1],
                        x1T[:, c, :], op0=ALU.add, op1=ALU.add)

        with (
            tc.tile_pool(name="expp", bufs=6) as exp_p,
            tc.tile_pool(name="exacc", bufs=3) as exacc_p,
            tc.tile_pool(name="sums", bufs=4) as sums_p,
            tc.tile_pool(name="wo_s", bufs=4) as wo_p,
            tc.tile_pool(name="ps_sc", bufs=2, space="PSUM") as ps_sc,
            tc.tile_pool(name="ps_at", bufs=2, space="PSUM") as ps_at,
            tc.tile_pool(name="ps_wo", bufs=4, space="PSUM") as ps_wo,
        ):
            for hg in range(4):
                for hh in range(8):
                    h = hg * 8 + hh
                    at_ps = ps_at.tile([P, T], f32, tag="at", name=f"at{h}")
                    ex_acc = exacc_p.tile([P, T], bf16, tag="exa",
                                          name=f"exa{h}")
                    wo_live = hg >= 1
                    if wo_live:
                        wbs, pss = wo_block(hg - 1, hh, ps_wo, wo_p)
                    exs = []
                    for sc in range(SC):
                        sc_ps = ps_sc.tile([P, T], f32, tag="sc",
                                           name=f"sc{h}_{sc}")
                        nc.tensor.matmul(sc_ps, kT[:, sc * P:(sc + 1) * P],
                                         q_tiles[h], start=True, stop=True)
                        ex = exp_p.tile([P, T], bf16, tag="ex",
                                        name=f"ex{h}_{sc}")
                        nc.scalar.activation(ex, sc_ps, AF.Exp,
                                             scale=invk_kv[:, sc:sc + 1],
                                             bias=kbias[:, sc, h:h + 1])
                        exs.append(ex)
                        if wo_live:
                            wo_mm(hg - 1, hh, wbs, pss, 2 * sc, q_tiles)
                            wo_mm(hg - 1, hh, wbs, pss, 2 * sc + 1, q_tiles)
                        if sc >= 1:
                            nc.tensor.matmul(at_ps, vtok[:, sc - 1, :],
                                             exs[sc - 1], start=(sc == 1),
                                             stop=False)
                        if sc == 0:
                            nc.vector.tensor_copy(ex_acc, ex)
                        else:
                            nc.vector.tensor_tensor(ex_acc, ex, ex_acc,
                                                    ALU.add)
                    nc.tensor.matmul(at_ps, vtok[:, SC - 1, :], exs[SC - 1],
                                     start=False, stop=True)
                    if wo_live:
                        wo_accum(hg - 1, hh, pss)
                    sum_bc = sums_p.tile([P, T], f32, tag="sum",
                                         name=f"sum{h}")
                    nc.gpsimd.partition_all_reduce(sum_bc, ex_acc, P, RED.add)
                    rec_bc = sums_p.tile([P, T], f32, tag="rec",
                                         name=f"rec{h}")
                    nc.vector.reciprocal_approx_fast(rec_bc, sum_bc)
                    nc.vector.tensor_tensor(q_tiles[h], at_ps, rec_bc,
                                            ALU.mult)
            # last head-group's Wo runs un-interleaved at the end
            for jg in range(8):
                wbs, pss = wo_block(3, jg, ps_wo, wo_p)
                for i in range(32):
                    wo_mm(3, jg, wbs, pss, i, q_tiles)
                wo_accum(3, jg, pss)

        # ---- Phase E: rmsnorm stats of x1 (ones-matmul partition reduce)
        p_head.release()
        kv_out.release()
        p_xtb.release()
        p_xn2 = tc.alloc_tile_pool(name="p_xn2", bufs=1)
        xn2T = p_xn2.tile([P, DC, T], bf16)
        with (
            tc.tile_pool(name="sq2", bufs=3) as sq2_p,
            tc.tile_pool(name="smalls2", bufs=2) as small2_p,
            tc.tile_pool(name="ps_ss", bufs=1, space="PSUM") as ps_ss,
        ):
            ssum = ps_ss.tile([1, T], f32, tag="ss2")
            for c in range(DC):
                sq = sq2_p.tile([P, T], bf16, tag="sq2", name=f"sq2_{c}")
                nc.vector.tensor_mul(sq, x1T[:, c, :], x1T[:, c, :])
                nc.tensor.matmul(ssum, ones_bf, sq, start=(c == 0),
                                 stop=(c == DC - 1))
            rms2 = small2_p.tile([1, T], f32, tag="rms2")
            nc.scalar.activation(rms2, ssum, AF.Sqrt, bias=eps_sb[:1, 0:1],
                                 scale=1.0 / D)
            inv2 = small2_p.tile([1, T], f32, tag="inv2")
            nc.vector.reciprocal_approx_fast(inv2, rms2)
            nc.gpsimd.partition_broadcast(nb2, inv2)
            for c in range(DC):
                nc.vector.tensor_mul(xn2T[:, c, :], x1T[:, c, :], nb2)

        # ---- Phase F: FFN, f-blocked, W2 accumulated into x1T in place
        with (
            tc.tile_pool(name="wf_s", bufs=8) as wf_p,
            tc.tile_pool(name="htp", bufs=16) as ht_p,
            tc.tile_pool(name="ps_w1", bufs=4, space="PSUM") as ps_w1,
            tc.tile_pool(name="ps_w2", bufs=4, space="PSUM") as ps_w2,
        ):
            for fb in range(NFB):
                ht_tiles = []
                for mg in range(2):
                    pss = [ps_w1.tile([P, T], f32, tag="w1",
                                      name=f"psw1_{fb}_{mg}_{j}")
                           for j in range(4)]
                    for kc4 in range(8):
                        wb = wf_p.tile([P, 4, 512], bf16, tag="wf")
                        nc.sync.dma_start(
                            wb, w1[kc4 * 512:(kc4 + 1) * 512,
                                   fb * FBLK + mg * 512:
                                   fb * FBLK + (mg + 1) * 512]
                            .rearrange("(k p) n -> p k n", p=P))
                        for k in range(4):
                            kc = kc4 * 4 + k
                            for j in range(4):
                                nc.tensor.matmul(pss[j],
                                                 wb[:, k, j * P:(j + 1) * P],
                                                 xn2T[:, kc, :],
                                                 start=(kc == 0),
                                                 stop=(kc == DC - 1))
                    for j in range(4):
                        m = fb * 8 + mg * 4 + j
                        ht = ht_p.tile([P, T], bf16, tag="ht", name=f"ht{m}")
                        nc.scalar.activation(ht, pss[j], AF.Gelu,
                                             bias=b1_sb[:, m:m + 1])
                        ht_tiles.append(ht)
                for jg in range(8):
                    pss = [ps_w2.tile([P, T], f32, tag="w2",
                                      name=f"psw2_{fb}_{jg}_{j}")
                           for j in range(4)]
                    for fq in range(2):
                        wb = wf_p.tile([P, 4, 512], bf16, tag="wf")
                        r0 = fb * FBLK + fq * 512
                        nc.sync.dma_start(
                            wb, w2[r0:r0 + 512, jg * 512:(jg + 1) * 512]
                            .rearrange("(k p) n -> p k n", p=P))
                        for k in range(4):
                            fc = fq * 4 + k
                            for j in range(4):
                                nc.tensor.matmul(pss[j],
                                                 wb[:, k, j * P:(j + 1) * P],
                                                 ht_tiles[fc],
                                                 start=(fc == 0),
                                                 stop=(fc == 7))
                    for j in range(4):
                        c = jg * 4 + j
                        if fb < NFB - 1:
                            nc.vector.tensor_tensor(x1T[:, c, :], pss[j],
                                                    x1T[:, c, :], ALU.add)
                        else:
                            nc.vector.scalar_tensor_tensor(
                                x1T[:, c, :], pss[j], b2_sb[:, c:c + 1],
                                x1T[:, c, :], op0=ALU.add, op1=ALU.add)
        p_xn2.release()

        # ---- Phase G: transpose back to token-major, store
        with (
            tc.tile_pool(name="outsl", bufs=2) as out_p,
            tc.tile_pool(name="ps_o", bufs=4, space="PSUM") as ps_o,
        ):
            for sub in range(4):
                osl = out_p.tile([P, D], f32, tag="osl", name=f"osl{sub}")
                for c in range(DC):
                    pt = ps_o.tile([P, P], f32, tag="tro",
                                   name=f"tro{sub}_{c}")
                    nc.tensor.transpose(pt, x1T[:, c, sub * P:(sub + 1) * P],
                                        ident)
                    nc.vector.tensor_copy(osl[:, c * P:(c + 1) * P], pt)
                nc.sync.dma_start(out[sub * P:(sub + 1) * P, :], osl)

        p_x1.release()
        consts.release()

    nc.compile()
    return nc


def get_program():
    if "nc" not in _CACHE:
        _CACHE["nc"] = _build_program()
    return _CACHE["nc"]


def make_in_maps(x, scale_attn, scale_ffn, Wq, bq, Wk, bk, Wv, bv, Wo, bo,
                 W1, b1, W2, b2):
    """Host-side prep: fold rmsnorm scales into weight rows, cast weights to
    bf16, build per-core rotated kv inputs."""
    import ml_dtypes

    f = np.float32
    bf = ml_dtypes.bfloat16
    sa = np.asarray(scale_attn, f)[:, None]
    sf = np.asarray(scale_ffn, f)[:, None]

    def pmaj(v, n):  # [n*128] -> [128, n]: partition-major bias layout
        return np.ascontiguousarray(np.asarray(v, f).reshape(n, P).T)

    def kvmaj(w):  # [4096, 128] -> [128, 32*128]: (p, c, k) layout
        w = np.asarray(w, f).reshape(DC, P, DK).transpose(1, 0, 2)
        return np.ascontiguousarray(w.reshape(P, DC * DK).astype(bf))

    shared = dict(
        wq=np.ascontiguousarray((np.asarray(Wq, f) * sa).astype(bf)),
        wk=kvmaj(np.asarray(Wk, f) * sa),
        wv=kvmaj(np.asarray(Wv, f) * sa),
        wo=np.ascontiguousarray(np.asarray(Wo, f).astype(bf)),
        w1=np.ascontiguousarray((np.asarray(W1, f) * sf).astype(bf)),
        w2=np.ascontiguousarray(np.asarray(W2, f).astype(bf)),
        bq=pmaj(bq, NH), bk=np.asarray(bk, f), bv=np.asarray(bv, f),
        bo=pmaj(bo, DC), b1=pmaj(b1, DFF // P), b2=pmaj(b2, DC),
    )
    xb = np.asarray(x, f).astype(bf)
    in_maps = []
    for c in range(NCORES):
        be, r0 = c // 4, (c % 4) * T
        x_rot = np.ascontiguousarray(np.roll(xb[be], -r0, axis=0))
        m = dict(shared)
        m["x_kv"] = x_rot
        in_maps.append(m)
    return in_maps


def kernel(**inputs):
    global LAST_RESULTS
    from concourse import bass_utils

    nc = get_program()
    in_maps = make_in_maps(**inputs)
    res = bass_utils.run_bass_kernel_spmd(nc, in_maps, core_ids=list(range(NCORES)))
    LAST_RESULTS = res
    x = np.asarray(inputs["x"], np.float32)
    out = np.empty_like(x)
    for c in range(NCORES):
        be, r0 = c // 4, (c % 4) * T
        out[be, r0:r0 + T, :] = res.results[c]["out"]
    return out
